# revision 10
# baseline (speedup 1.0000x reference)
"""AugLag bit-decomposed linear layer on 8 Trainium2 NeuronCores.

Computes out = x @ w.T + b where w = sum_k(w_twos[..., k] * base[k]) * step.

Sharding: tensor-parallel over output features. Each of the 8 cores gets a
[512, 4096, 8] slice of w_twos (passed bit-plane-major as [8, 4096, 512]),
the full x (pre-transposed on host to [4096, 8192] so the contraction dim
lands on SBUF partitions), and its bias slice.

Per-core pipeline (measured on HW: f32r matmul 307ns, bf16 252ns, fp8
DoubleRow 253ns per 2 K-blocks):
- w2p streams in on the Act+Pool DGE queues (sync queue is reserved for x),
  DVE reconstructs w.T via a 7-op Horner chain per 128-row block, writing
  bf16 weight tiles (fp8e4 pair tiles for the last FP8_PAIRS*2 blocks).
- x streams on the sync queue; the Act engine casts it to bf16 (fp8e4 for
  the DoubleRow blocks). Chunk 0's casts are interleaved into Act's w2p
  stream so the PE starts ~15us in.
- TensorE: bf16 matmuls (stationary x subtile, moving weight tile), plus
  fp8 DoubleRow matmuls covering 2 K-blocks each for the fp8 pairs.
- Drain on DVE applies *step and +bias; outputs leave via gpsimd SWDGE.

Accuracy: bf16 x (~2e-3) + bf16 w (~1.2e-3) + fp8 on 4/32 of the
contraction (~1.3e-2) => ~1.3e-2 max rel error vs the 2e-2 gate.
"""

import os

import numpy as np

import concourse.mybir as mybir
import concourse.tile as tile
from concourse import bacc
from concourse.bass_utils import run_bass_kernel_spmd

N_CORES = 8
N_TOK = 8192
IN_F = 4096
OUT_F = 4096
N_BITS = 8
STEP_SIZE = 0.0078125
OF_SH = OUT_F // N_CORES  # 512 output features per core

P = 128
FP8_PAIRS = 2  # trailing K-block pairs computed in fp8 DoubleRow (2 kb each)


def build_program(base_vals, n_tok=N_TOK, in_f=IN_F, of_sh=OF_SH, n_bits=N_BITS,
                  step=STEP_SIZE, n_repeat=1, fp8_pairs=FP8_PAIRS):
    """Build the per-core Bass program (SPMD: same program on all cores)."""
    f32 = mybir.dt.float32
    bf16 = mybir.dt.bfloat16
    f8 = mybir.dt.float8e4
    KB = in_f // P      # contraction blocks of 128
    TT = n_tok // P     # output row tiles of 128

    nc = bacc.Bacc(None, target_bir_lowering=False, debug=False)
    xT = nc.declare_dram_parameter("xT", [in_f, n_tok], f32, isOutput=False)
    w2p = nc.declare_dram_parameter("w2p", [n_bits, in_f, of_sh], f32,
                                    isOutput=False)
    bias = nc.declare_dram_parameter("bias", [P, of_sh], f32, isOutput=False)
    out = nc.declare_dram_parameter("out", [n_tok, of_sh], f32, isOutput=True)

    with tile.TileContext(nc) as tc:
        with (
            tc.tile_pool(name="wt", bufs=1) as wtp,
            tc.tile_pool(name="stage", bufs=2) as stp,
            tc.tile_pool(name="xc", bufs=3) as xcp,
            tc.tile_pool(name="xb", bufs=3) as xbp,
            tc.tile_pool(name="ob", bufs=3) as obp,
            tc.tile_pool(name="cst", bufs=1) as cst,
            tc.tile_pool(name="ps", bufs=8, space="PSUM") as psp,
        ):
            bias_t = cst.tile([P, of_sh], f32)
            nc.sync.dma_start(bias_t[:], bias[:])

            for _rep in range(n_repeat):
                _emit_body(nc, tc, xT, w2p, out, bias_t, base_vals, step,
                           KB, TT, n_bits, of_sh, fp8_pairs,
                           wtp, stp, xcp, xbp, obp, psp, f32, bf16, f8)

    nc.compile()
    return nc


def _emit_body(nc, tc, xT, w2p, out, bias_t, base_vals, step, KB, TT, n_bits,
               of_sh, fp8_pairs, wtp, stp, xcp, xbp, obp, psp, f32, bf16, f8):
    T_CHUNK = 1024 if (TT * P) % 1024 == 0 else P
    TS = T_CHUNK // P                 # token subtiles per chunk
    TC = (TT * P) // T_CHUNK          # number of token chunks
    KBQ = 4 if (KB % 4 == 0 and T_CHUNK > P) else KB  # kb blocks per x tile
    NQ = KB // KBQ
    OB_TS = min(TS, 4)                # token subtiles per output write
    # fp8 DoubleRow covers the trailing kb blocks; they must fill whole
    # KBQ-quarters for the x-cast tiling below.
    n_fp8 = 2 * fp8_pairs
    assert n_fp8 % KBQ == 0 or n_fp8 == 0 or KBQ % n_fp8 == 0
    FP8_KB0 = KB - n_fp8
    xTr = xT.rearrange("(kb p) t -> p kb t", p=P)

    def load_xq(tcc, q):
        xq = xcp.tile([P, KBQ, T_CHUNK], f32, tag="xq", name=f"xq{tcc}_{q}")
        tsl = slice(tcc * T_CHUNK, (tcc + 1) * T_CHUNK)
        nc.sync.dma_start(xq[:], xTr[:, q * KBQ:(q + 1) * KBQ, tsl])
        return xq

    def cast_x(tcc, q, xq):
        """Act engine: cast an x quarter to bf16 (or fp8e4 for DR blocks)."""
        if q * KBQ >= FP8_KB0:
            xb = xbp.tile([P, KBQ, T_CHUNK], f8, tag="x8", name=f"x8_{tcc}_{q}")
        else:
            xb = xbp.tile([P, KBQ, T_CHUNK], bf16, tag="xb",
                          name=f"xb{tcc}_{q}")
        nc.scalar.copy(xb[:], xq[:])
        return xb

    # Phase A: reconstruct w.T on DVE via Horner (7 ops/kb; the first op
    # consumes two planes, the trailing base factor folds into the drain
    # scale). w2p stream alternates Act/Pool queues. Chunk-0 x loads+casts
    # are interleaved into the emission so the PE can start early.
    horner = all(abs(float(b)) > 1e-30 for b in base_vals)
    wts = []
    wt8 = [wtp.tile([P, 2, of_sh], f8, tag=f"wt8_{m}", name=f"wt8_{m}")
           for m in range(fp8_pairs)]
    pre = {}
    for kb in range(KB):
        st = stp.tile([P, n_bits, of_sh], f32, tag="stage")
        qeng = nc.scalar if kb % 2 == 0 else nc.gpsimd
        qeng.dma_start(
            st[:],
            w2p[:, kb * P:(kb + 1) * P, :].rearrange("k p o -> p k o"),
        )
        if kb >= FP8_KB0:
            pair = (kb - FP8_KB0) // 2
            wt = wt8[pair][:, (kb - FP8_KB0) % 2, :]
        else:
            wtt = wtp.tile([P, of_sh], bf16, tag=f"wt{kb}")
            wts.append(wtt)
            wt = wtt[:]
        acc = stp.tile([P, of_sh], f32, tag="acc")
        if horner:
            nc.vector.scalar_tensor_tensor(
                acc[:], st[:, 0, :], float(base_vals[0] / base_vals[1]),
                st[:, 1, :], mybir.AluOpType.mult, mybir.AluOpType.add,
            )
            for k in range(2, n_bits):
                dst = wt if k == n_bits - 1 else acc[:]
                nc.vector.scalar_tensor_tensor(
                    dst, acc[:], float(base_vals[k - 1] / base_vals[k]),
                    st[:, k, :], mybir.AluOpType.mult, mybir.AluOpType.add,
                )
        else:
            nc.vector.tensor_scalar_mul(acc[:], st[:, 0, :], float(base_vals[0]))
            for k in range(1, n_bits - 1):
                nc.vector.scalar_tensor_tensor(
                    acc[:], st[:, k, :], float(base_vals[k]), acc[:],
                    mybir.AluOpType.mult, mybir.AluOpType.add,
                )
            nc.vector.scalar_tensor_tensor(
                wt, st[:, n_bits - 1, :], float(base_vals[n_bits - 1]),
                acc[:], mybir.AluOpType.mult, mybir.AluOpType.add,
            )
        # chunk-0 x prep, interleaved so Act alternates w2p DMAs and casts
        if NQ and (kb + 1) % (KB // NQ) == 0:
            q = (kb + 1) // (KB // NQ) - 1
            if q < NQ:
                pre[q] = cast_x(0, q, load_xq(0, q))

    step_eff = step * float(base_vals[-1]) if horner else step
    outr = out.rearrange("(c j p) o -> c p j o", p=P, j=OB_TS)
    outr1 = out.rearrange("(c p) o -> c p o", p=P)
    for tcc in range(TC):
        pss = [psp.tile([P, of_sh], f32, name=f"ps{j}", tag="ps")
               for j in range(TS)]
        for q in range(NQ):
            if tcc == 0 and q in pre:
                xb = pre[q]
            else:
                xb = cast_x(tcc, q, load_xq(tcc, q))
            if (q + 1) * KBQ <= FP8_KB0:
                for kbq in range(KBQ):
                    kb = q * KBQ + kbq
                    for j in range(TS):
                        nc.tensor.matmul(
                            pss[j][:],
                            xb[:, kbq, j * P:(j + 1) * P],
                            wts[kb][:],
                            start=(kb == 0),
                            stop=False,
                        )
            else:
                # fp8 DoubleRow: each matmul contracts a kb PAIR
                for kbq2 in range(KBQ // 2):
                    kb = q * KBQ + 2 * kbq2
                    pair = (kb - FP8_KB0) // 2
                    last = kb + 2 == KB
                    for j in range(TS):
                        nc.tensor.matmul(
                            pss[j][:],
                            xb[:, 2 * kbq2:2 * kbq2 + 2, j * P:(j + 1) * P],
                            wt8[pair][:, :, :],
                            start=False,
                            stop=last,
                            perf_mode=mybir.MatmulPerfMode.DoubleRow,
                            skip_group_check=True,
                        )
        # SWDGE (gpsimd) output path keeps writes off the sync x queue.
        # The final chunk drains per-subtile to shorten the kernel tail.
        if tcc == TC - 1:
            for j in range(TS):
                ob = obp.tile([P, of_sh], f32, tag="ob1", name=f"obl{j}")
                nc.vector.scalar_tensor_tensor(
                    ob[:], pss[j][:], float(step_eff), bias_t[:],
                    mybir.AluOpType.mult, mybir.AluOpType.add,
                )
                nc.gpsimd.dma_start(outr1[tcc * TS + j], ob[:])
        else:
            for h in range(TS // OB_TS):
                ob = obp.tile([P, OB_TS, of_sh], f32, tag="ob", name=f"ob{h}")
                for j in range(OB_TS):
                    nc.vector.scalar_tensor_tensor(
                        ob[:, j, :], pss[h * OB_TS + j][:], float(step_eff),
                        bias_t[:],
                        mybir.AluOpType.mult, mybir.AluOpType.add,
                    )
                nc.gpsimd.dma_start(outr[tcc * (TS // OB_TS) + h], ob[:])


_program_cache = {}


def _get_program(base_vals):
    key = tuple(base_vals)
    if key not in _program_cache:
        _program_cache[key] = build_program(base_vals)
    return _program_cache[key]


def prep_in_maps(x, w_twos, b):
    """Host-side layout prep (no arithmetic): transpose + shard + replicate."""
    xT = np.ascontiguousarray(x.T)  # [IN_F, N_TOK]
    in_maps = []
    for c in range(N_CORES):
        sl = slice(c * OF_SH, (c + 1) * OF_SH)
        # [OF_SH, IN_F, N_BITS] -> bit-plane-major [N_BITS, IN_F, OF_SH]
        w2p = np.ascontiguousarray(w_twos[sl].transpose(2, 1, 0))
        bias = np.ascontiguousarray(
            np.broadcast_to(b[sl][None, :], (P, OF_SH)))
        in_maps.append({"xT": xT, "w2p": w2p, "bias": bias})
    return in_maps


def kernel(x, w_twos, b, base, **_kwargs):
    x = np.asarray(x, dtype=np.float32)
    w_twos = np.asarray(w_twos, dtype=np.float32)
    b = np.asarray(b, dtype=np.float32)
    base_vals = [float(v) for v in np.asarray(base, dtype=np.float32).reshape(-1)]

    nc = _get_program(base_vals)
    in_maps = prep_in_maps(x, w_twos, b)

    kwargs = {}
    if os.environ.get("KERNEL_TRACE"):
        kwargs["trace"] = True
        if os.environ.get("KERNEL_TRACE_DIR"):
            kwargs["tmpdir"] = os.environ["KERNEL_TRACE_DIR"]
    res = run_bass_kernel_spmd(nc, in_maps, list(range(N_CORES)), **kwargs)
    globals()["last_results"] = res
    out = np.concatenate([res.results[c]["out"] for c in range(N_CORES)],
                         axis=1)
    return out
